# revision 4
# baseline (speedup 1.0000x reference)
"""Trainium2 Bass kernel for nn_Explore_decoder_add (histogram_binning).

Strategy (8 NeuronCores, tensor-parallel on vocab):
  - The attention-pooling part (tiny) is replicated on every core.
  - Wec/bec and the (B, V) logits are sharded over vocab: 12500 cols/core
    (padded to 12544 = 98*128).
  - Output layout on device is [v_part(128), b(16), c(98)] (vocab chunk on
    partitions) so the epilogue (mask add, exp, reduce) runs on 128 lanes.
  - The histogram "seen-id" mask is computed with per-batch one-hot matmuls
    on the tensor engine (exactly the bmm(mask, one_hot) formulation,
    restricted to the local vocab shard via p = lv%128 / c = lv>>7).
  - Distributed softmax: per-core exp sums are AllReduce'd (add) across the
    8 cores, then each core scales its shard by 1/total.
  - The big Wec matmul streams Wec through LDWEIGHTS as bf16 hi/lo splits
    (4-term product reconstructs fp32 to ~2^-18 relative), with the small
    [h_t; c_s] operand also hi/lo split and packed into the moving operand.

Host side only shards/pads/re-encodes inputs and unshards the output.
"""

import numpy as np
import ml_dtypes

B, S, D = 16, 200, 128
V = 100000
NCORES = 8
VS = V // NCORES            # 12500 vocab per core
NCHUNK = 98                 # 98 chunks of 128
VSP = NCHUNK * 128          # 12544 padded shard width
SCH0, SCH1 = 128, 72        # token chunks per batch (200 = 128 + 72)
SPAD = 256                  # padded per-batch token stride
NEG = -1.0e30

# main-stream grouping
DMA_G = 14                  # W chunks per DMA group   (98 = 7 * 14)
PS_G = 7                    # W chunks per PSUM group  (98 = 14 * 7)
N_DMA_G = NCHUNK // DMA_G
N_PS_G = NCHUNK // PS_G

_prog_cache = {}


def _build_program():
    import concourse.bacc as bacc
    import concourse.mybir as mybir
    import concourse.tile as tile
    from concourse.masks import make_identity

    f32 = mybir.dt.float32
    bf16 = mybir.dt.bfloat16
    i32 = mybir.dt.int32
    OP = mybir.AluOpType
    ACT = mybir.ActivationFunctionType

    nc = bacc.Bacc("TRN2", target_bir_lowering=False, debug=False,
                   num_devices=NCORES)

    # ---- I/O -------------------------------------------------------------
    x = nc.dram_tensor("x", (B, S, D), f32, kind="ExternalInput").ap()
    ids = nc.dram_tensor("x_ids", (B, S), i32, kind="ExternalInput").ap()
    wq = nc.dram_tensor("Wq", (D, D), f32, kind="ExternalInput").ap()
    bq = nc.dram_tensor("bq", (D,), f32, kind="ExternalInput").ap()
    wk = nc.dram_tensor("Wk", (D, D), f32, kind="ExternalInput").ap()
    bk = nc.dram_tensor("bk", (D,), f32, kind="ExternalInput").ap()
    wv = nc.dram_tensor("Wv", (D, 1), f32, kind="ExternalInput").ap()
    whi0 = nc.dram_tensor("whi0", (D, VSP), bf16, kind="ExternalInput").ap()
    wlo0 = nc.dram_tensor("wlo0", (D, VSP), bf16, kind="ExternalInput").ap()
    whi1 = nc.dram_tensor("whi1", (D, VSP), bf16, kind="ExternalInput").ap()
    wlo1 = nc.dram_tensor("wlo1", (D, VSP), bf16, kind="ExternalInput").ap()
    becp = nc.dram_tensor("becp", (VSP,), f32, kind="ExternalInput").ap()
    lo_vec = nc.dram_tensor("lo_vec", (128, 1), f32, kind="ExternalInput").ap()
    out = nc.dram_tensor("out", (128, B * NCHUNK), f32,
                         kind="ExternalOutput").ap()

    with tile.TileContext(nc) as tc:
        with (
            tc.tile_pool(name="sb", bufs=1) as sb,
            tc.tile_pool(name="wpool", bufs=2) as wpool,
            tc.tile_pool(name="ohpool", bufs=3) as ohpool,
            tc.tile_pool(name="scpool", bufs=2) as scpool,
            tc.tile_pool(name="pp", bufs=2, space="PSUM") as pp,
            tc.tile_pool(name="ph", bufs=2, space="PSUM") as ph,
            tc.tile_pool(name="pm", bufs=3, space="PSUM") as pm,
            tc.tile_pool(name="pt", bufs=1, space="PSUM") as pt,
            tc.tile_pool(name="dram", bufs=1, space="DRAM") as dram,
        ):
            # ---- constants & small loads --------------------------------
            ident = sb.tile([128, 128], f32, name="ident")
            make_identity(nc, ident[:, :])
            ones_col = sb.tile([128, 1], f32, name="ones_col")
            nc.gpsimd.memset(ones_col[:, :], 1.0)
            ones_row = sb.tile([1, 128], f32, name="ones_row")
            nc.gpsimd.memset(ones_row[:, :], 1.0)

            iota_p_i = sb.tile([128, 128], i32, name="iota_p_i")
            nc.gpsimd.iota(iota_p_i[:, :], pattern=[[1, 128]],
                           channel_multiplier=0)
            iota_c_i = sb.tile([128, NCHUNK], i32, name="iota_c_i")
            nc.gpsimd.iota(iota_c_i[:, :], pattern=[[1, NCHUNK]],
                           channel_multiplier=0)
            iota_p = sb.tile([128, 128], f32, name="iota_p")
            nc.vector.tensor_copy(iota_p[:, :], iota_p_i[:, :])
            iota_c = sb.tile([128, NCHUNK], f32, name="iota_c")
            nc.vector.tensor_copy(iota_c[:, :], iota_c_i[:, :])

            wq_sb = sb.tile([D, D], f32, name="wq_sb")
            nc.sync.dma_start(out=wq_sb[:, :], in_=wq[:, :])
            wk_sb = sb.tile([D, D], f32, name="wk_sb")
            nc.sync.dma_start(out=wk_sb[:, :], in_=wk[:, :])
            wv_sb = sb.tile([D, 1], f32, name="wv_sb")
            nc.sync.dma_start(out=wv_sb[:, :], in_=wv[:, :])
            bq_sb = sb.tile([D, 1], f32, name="bq_sb")
            nc.sync.dma_start(out=bq_sb[:, :], in_=bq[:, None])
            bk_sb = sb.tile([D, 1], f32, name="bk_sb")
            nc.sync.dma_start(out=bk_sb[:, :], in_=bk[:, None])
            lo_sb = sb.tile([128, 1], f32, name="lo_sb")
            nc.sync.dma_start(out=lo_sb[:, :], in_=lo_vec[:, :])
            bec_sb = sb.tile([128, NCHUNK], f32, name="bec_sb")
            nc.sync.dma_start(out=bec_sb[:, :],
                              in_=becp.rearrange("(c p) -> p c", p=128))

            # x, batch-aligned: X0 = x[:, 0:128, :], X1 = x[:, 128:200, :]
            X0 = sb.tile([128, B, D], f32, name="X0")
            nc.sync.dma_start(out=X0[:, :, :],
                              in_=x[:, 0:SCH0, :].transpose([1, 0, 2]))
            X1 = sb.tile([128, B, D], f32, name="X1")
            nc.sync.dma_start(out=X1[0:SCH1, :, :],
                              in_=x[:, SCH0:S, :].transpose([1, 0, 2]))
            ids0 = sb.tile([128, B], i32, name="ids0")
            nc.sync.dma_start(out=ids0[:, :],
                              in_=ids[:, 0:SCH0].transpose([1, 0]))
            ids1 = sb.tile([128, B], i32, name="ids1")
            nc.sync.dma_start(out=ids1[0:SCH1, :],
                              in_=ids[:, SCH0:S].transpose([1, 0]))

            # ---- transposes: x -> xT_pad [d, b, s(256)] ------------------
            xT = sb.tile([128, B, SPAD], f32, name="xT")
            for b in range(B):
                for ci, xsrc in ((0, X0), (1, X1)):
                    tps = pp.tile([128, 128], f32, name="tps", tag="pp")
                    nc.tensor.transpose(out=tps[:, :], in_=xsrc[:, b, :],
                                        identity=ident[:, :])
                    eng = nc.vector if (b + ci) % 2 == 0 else nc.scalar
                    if eng is nc.vector:
                        nc.vector.tensor_copy(
                            xT[:, b, ci * 128:(ci + 1) * 128], tps[:, :])
                    else:
                        nc.scalar.copy(
                            xT[:, b, ci * 128:(ci + 1) * 128], tps[:, :])

            x0T = sb.tile([128, B], f32, name="x0T")
            nc.vector.tensor_copy(
                x0T[:, :], xT[:, :, 0:1].rearrange("p b one -> p (b one)"))

            # ---- k^T + combined bias ------------------------------------
            bias_eq = sb.tile([128, 1], f32, name="bias_eq")
            nc.vector.tensor_tensor(out=bias_eq[:, :], in0=bq_sb[:, :],
                                    in1=bk_sb[:, :], op=OP.add)
            kps = pp.tile([128, B], f32, name="kps", tag="pp")
            nc.tensor.matmul(out=kps[:, :], lhsT=wk_sb[:, :], rhs=x0T[:, :],
                             start=True, stop=True)
            kTb = sb.tile([128, B], f32, name="kTb")
            nc.vector.tensor_scalar(kTb[:, :], kps[:, :], bias_eq[:, 0:1],
                                    None, OP.add)

            # ---- q^T (+ tanh fused via ACT bias) -> fT -------------------
            fT = sb.tile([128, B, SPAD], f32, name="fT")
            xTf = xT.rearrange("p b s -> p (b s)")
            for g in range(8):  # 8 groups of 512 cols (= 2 batches each)
                qps = pp.tile([128, 512], f32, name="qps", tag="pp")
                nc.tensor.matmul(out=qps[:, :], lhsT=wq_sb[:, :],
                                 rhs=xTf[:, g * 512:(g + 1) * 512],
                                 start=True, stop=True)
                for j in range(2):
                    b = 2 * g + j
                    nc.scalar.activation(
                        out=fT[:, b, :], in_=qps[:, j * SPAD:(j + 1) * SPAD],
                        func=ACT.Tanh, bias=kTb[:, b:b + 1])

            # ---- scores = Wv^T @ fT -> [1, 4096] -> [16, 200] ------------
            scores_row = sb.tile([1, B * SPAD], f32, name="scores_row")
            fTf = fT.rearrange("p b s -> p (b s)")
            for g in range(8):
                sps = pp.tile([1, 512], f32, name="sps", tag="pp")
                nc.tensor.matmul(out=sps[:, :], lhsT=wv_sb[:, :],
                                 rhs=fTf[:, g * 512:(g + 1) * 512],
                                 start=True, stop=True)
                nc.scalar.copy(scores_row[:, g * 512:(g + 1) * 512],
                               sps[:, :])

            sc_dram = dram.tile([B, S], f32, name="sc_dram")
            sc_view = scores_row.rearrange("p (b s) -> p b s", b=B)
            nc.sync.dma_start(out=sc_dram[:, :], in_=sc_view[:, :, 0:S])
            scT = sb.tile([B, S], f32, name="scT")
            nc.sync.dma_start(out=scT[:, :], in_=sc_dram[:, :])

            # softmax over s (per batch row)
            rmax = sb.tile([B, 1], f32, name="rmax")
            nc.vector.tensor_reduce(out=rmax[:, :], in_=scT[:, :],
                                    axis=mybir.AxisListType.X, op=OP.max)
            negmax = sb.tile([B, 1], f32, name="negmax")
            nc.vector.tensor_scalar(negmax[:, :], rmax[:, :], -1.0, None,
                                    OP.mult)
            e_s = sb.tile([B, S], f32, name="e_s")
            ssum = sb.tile([B, 1], f32, name="ssum")
            nc.scalar.activation(out=e_s[:, :], in_=scT[:, :], func=ACT.Exp,
                                 bias=negmax[:, 0:1], accum_out=ssum[:, :])
            sinv = sb.tile([B, 1], f32, name="sinv")
            nc.vector.reciprocal(sinv[:, :], ssum[:, :])
            probs = sb.tile([B, S], f32, name="probs")
            nc.vector.tensor_scalar(probs[:, :], e_s[:, :], sinv[:, 0:1],
                                    None, OP.mult)

            # transpose probs -> [s, b] (two chunks)
            s_sT0 = sb.tile([128, B], f32, name="s_sT0")
            tp0 = pp.tile([128, B], f32, name="tp0", tag="pp")
            nc.tensor.transpose(out=tp0[:, :], in_=probs[:, 0:128],
                                identity=ident[0:B, 0:B])
            nc.vector.tensor_copy(s_sT0[:, :], tp0[:, :])
            s_sT1 = sb.tile([128, B], f32, name="s_sT1")
            tp1 = pp.tile([SCH1, B], f32, name="tp1", tag="pp")
            nc.tensor.transpose(out=tp1[:, :], in_=probs[:, 128:200],
                                identity=ident[0:B, 0:B])
            nc.vector.tensor_copy(s_sT1[0:SCH1, :], tp1[:, :])

            # ---- c_s^T = sum_s x[b,s,:] * probs[b,s]  -> [d, b] ----------
            csps = pp.tile([128, B], f32, name="csps", tag="pp")
            for b in range(B):
                nc.tensor.matmul(out=csps[:, b:b + 1], lhsT=X0[:, b, :],
                                 rhs=s_sT0[:, b:b + 1], start=True,
                                 stop=False)
                nc.tensor.matmul(out=csps[:, b:b + 1],
                                 lhsT=X1[0:SCH1, b, :],
                                 rhs=s_sT1[0:SCH1, b:b + 1], start=False,
                                 stop=True)
            csT = sb.tile([128, B], f32, name="csT")
            nc.vector.tensor_copy(csT[:, :], csps[:, :])

            # ---- hi/lo split of [x0T | csT] into moving operand v4 -------
            v4 = sb.tile([128, 4 * B], bf16, name="v4")
            res = sb.tile([128, B], f32, name="res")
            for i, src in enumerate((x0T, csT)):
                nc.vector.tensor_copy(v4[:, (2 * i) * B:(2 * i + 1) * B],
                                      src[:, :])
                nc.vector.tensor_tensor(
                    out=res[:, :], in0=src[:, :],
                    in1=v4[:, (2 * i) * B:(2 * i + 1) * B], op=OP.subtract)
                nc.vector.tensor_copy(v4[:, (2 * i + 1) * B:(2 * i + 2) * B],
                                      res[:, :])

            # ---- histogram mask -> additive penalty ----------------------
            # penalty[p, b, c] = bec[p, c] + (-1e30) * count_b(v = c*128+p)
            penalty = sb.tile([128, B, NCHUNK], f32, name="penalty")
            nc.vector.tensor_copy(
                penalty[:, :, :],
                bec_sb.unsqueeze(1).broadcast_to([128, B, NCHUNK]))

            prep = []
            for idt in (ids0, ids1):
                idf = scpool.tile([128, B], f32, name="idf", tag="idf")
                nc.vector.tensor_copy(idf[:, :], idt[:, :])
                lv = scpool.tile([128, B], f32, name="lv", tag="lv")
                nc.vector.tensor_scalar(lv[:, :], idf[:, :], lo_sb[:, 0:1],
                                        None, OP.subtract)
                # c = floor(lv/128) via round-to-nearest(lv/128 - 0.4999)
                # (lv is integer-valued, |lv| < 1e5, so the offset is safe)
                ct = scpool.tile([128, B], f32, name="ct", tag="ct")
                nc.vector.tensor_scalar(ct[:, :], lv[:, :], 1.0 / 128.0,
                                        -0.4999, OP.mult, OP.add)
                ci = scpool.tile([128, B], i32, name="ci", tag="ci")
                nc.vector.tensor_copy(ci[:, :], ct[:, :])
                c_f = scpool.tile([128, B], f32, name="c_f", tag="c_f")
                nc.vector.tensor_copy(c_f[:, :], ci[:, :])
                # p = lv - 128*c
                p_f = scpool.tile([128, B], f32, name="p_f", tag="p_f")
                nc.vector.tensor_scalar(p_f[:, :], c_f[:, :], -128.0, None,
                                        OP.mult)
                nc.vector.tensor_tensor(out=p_f[:, :], in0=p_f[:, :],
                                        in1=lv[:, :], op=OP.add)
                bad = scpool.tile([128, B], f32, name="bad", tag="bad")
                nc.vector.tensor_scalar(bad[:, :], idf[:, :], 1.5, 1000.0,
                                        OP.is_lt, OP.mult)
                p_use = scpool.tile([128, B], f32, name="p_use", tag="pu",
                                    bufs=2)
                nc.vector.tensor_tensor(out=p_use[:, :], in0=p_f[:, :],
                                        in1=bad[:, :], op=OP.add)
                prep.append((p_use, c_f))

            for b in range(B):
                hps = ph.tile([128, NCHUNK], f32, name="hps", tag="ph")
                for ci, (p_use, c_f) in enumerate(prep):
                    np_ = 128 if ci == 0 else SCH1
                    ohp = ohpool.tile([128, 128], bf16, name="ohp",
                                      tag="ohp")
                    nc.vector.tensor_scalar(ohp[:, :], iota_p[:, :],
                                            p_use[:, b:b + 1], NEG,
                                            OP.is_equal, OP.mult)
                    ohc = ohpool.tile([128, NCHUNK], bf16, name="ohc",
                                      tag="ohc")
                    nc.vector.tensor_scalar(ohc[:, :], iota_c[:, :],
                                            c_f[:, b:b + 1], None,
                                            OP.is_equal)
                    nc.tensor.matmul(out=hps[:, :], lhsT=ohp[0:np_, :],
                                     rhs=ohc[0:np_, :], start=(ci == 0),
                                     stop=(ci == 1))
                nc.vector.tensor_tensor(out=penalty[:, b, :],
                                        in0=penalty[:, b, :], in1=hps[:, :],
                                        op=OP.add)

            # ---- main stream: logits -> masked exp -----------------------
            exp_buf = sb.tile([128, B, NCHUNK], f32, name="exp_buf")
            w_tiles = {}
            for name, t in (("whi0", whi0), ("wlo0", wlo0), ("whi1", whi1),
                            ("wlo1", wlo1)):
                w_tiles[name] = (t, None)

            for dg in range(N_DMA_G):
                c0 = dg * DMA_G
                cur = {}
                for name, (t, _) in w_tiles.items():
                    wt = wpool.tile([128, DMA_G * 128], bf16, name=name,
                                    tag=name)
                    nc.sync.dma_start(
                        out=wt[:, :],
                        in_=t[:, c0 * 128:(c0 + DMA_G) * 128])
                    cur[name] = wt
                for pg in range(DMA_G // PS_G):
                    ps = pm.tile([128, PS_G, 2, B], f32, name="ps", tag="pm")
                    for j in range(PS_G):
                        cc = pg * PS_G + j          # chunk within DMA group
                        sl = slice(cc * 128, (cc + 1) * 128)
                        nc.tensor.matmul(out=ps[:, j, :, :],
                                         lhsT=cur["whi0"][:, sl],
                                         rhs=v4[:, 0:2 * B], start=True,
                                         stop=False)
                        nc.tensor.matmul(out=ps[:, j, :, :],
                                         lhsT=cur["wlo0"][:, sl],
                                         rhs=v4[:, 0:2 * B], start=False,
                                         stop=False)
                        nc.tensor.matmul(out=ps[:, j, :, :],
                                         lhsT=cur["whi1"][:, sl],
                                         rhs=v4[:, 2 * B:4 * B], start=False,
                                         stop=False)
                        nc.tensor.matmul(out=ps[:, j, :, :],
                                         lhsT=cur["wlo1"][:, sl],
                                         rhs=v4[:, 2 * B:4 * B], start=False,
                                         stop=True)
                    g7 = slice(c0 + pg * PS_G, c0 + (pg + 1) * PS_G)
                    scr = scpool.tile([128, PS_G, B], f32, name="scr",
                                      tag="scr")
                    nc.vector.tensor_tensor(
                        out=scr[:, :, :], in0=ps[:, :, 0, :],
                        in1=penalty[:, :, g7].transpose([0, 2, 1]),
                        op=OP.add)
                    nc.vector.tensor_tensor(
                        out=exp_buf[:, :, g7].transpose([0, 2, 1]),
                        in0=scr[:, :, :], in1=ps[:, :, 1, :], op=OP.add)
                    nc.scalar.activation(out=exp_buf[:, :, g7],
                                         in_=exp_buf[:, :, g7],
                                         func=ACT.Exp)

            # ---- distributed softmax denominator -------------------------
            partials = sb.tile([128, B], f32, name="partials")
            nc.vector.tensor_reduce(out=partials[:, :],
                                    in_=exp_buf[:, :, :],
                                    axis=mybir.AxisListType.X, op=OP.add)
            tot_ps = pt.tile([1, B], f32, name="tot_ps", tag="pt")
            nc.tensor.matmul(out=tot_ps[:, :], lhsT=ones_col[:, :],
                             rhs=partials[:, :], start=True, stop=True)
            sums_sb = sb.tile([1, B], f32, name="sums_sb")
            nc.vector.tensor_copy(sums_sb[:, :], tot_ps[:, :])

            ccin = dram.tile([1, B], f32, name="ccin")
            ccout = dram.tile([1, B], f32, name="ccout")
            nc.sync.dma_start(out=ccin[:, :], in_=sums_sb[:, :])
            nc.gpsimd.collective_compute(
                "AllReduce",
                OP.add,
                replica_groups=[list(range(NCORES))],
                ins=[ccin.opt()],
                outs=[ccout.opt()],
            )
            gsum = sb.tile([1, B], f32, name="gsum")
            nc.sync.dma_start(out=gsum[:, :], in_=ccout[:, :])
            ginv = sb.tile([1, B], f32, name="ginv")
            nc.vector.reciprocal(ginv[:, :], gsum[:, :])

            inv_ps = pt.tile([128, B], f32, name="inv_ps", tag="pt")
            nc.tensor.matmul(out=inv_ps[:, :], lhsT=ones_row[:, :],
                             rhs=ginv[:, :], start=True, stop=True)
            nc.vector.tensor_tensor(
                out=exp_buf[:, :, :], in0=exp_buf[:, :, :],
                in1=inv_ps.unsqueeze(2).broadcast_to([128, B, NCHUNK]),
                op=OP.mult)

            nc.sync.dma_start(out=out[:, :],
                              in_=exp_buf.rearrange("p b c -> p (b c)"))

    nc.compile()
    return nc


def _get_program():
    if "nc" not in _prog_cache:
        _prog_cache["nc"] = _build_program()
    return _prog_cache["nc"]


def kernel(x, x_ids, Wq, bq, Wk, bk, Wv, bv, Wec, bec):
    bf16 = ml_dtypes.bfloat16
    x = np.ascontiguousarray(np.asarray(x, dtype=np.float32))
    ids = np.ascontiguousarray(np.asarray(x_ids).astype(np.int32))
    Wq = np.ascontiguousarray(np.asarray(Wq, dtype=np.float32))
    bq = np.ascontiguousarray(np.asarray(bq, dtype=np.float32))
    Wk = np.ascontiguousarray(np.asarray(Wk, dtype=np.float32))
    bk = np.ascontiguousarray(np.asarray(bk, dtype=np.float32))
    Wv = np.ascontiguousarray(np.asarray(Wv, dtype=np.float32))
    Wec = np.asarray(Wec, dtype=np.float32)
    bec = np.asarray(bec, dtype=np.float32)

    nc = _get_program()

    in_maps = []
    for r in range(NCORES):
        lo, hi = r * VS, (r + 1) * VS
        wp = np.zeros((2 * D, VSP), np.float32)
        wp[:, :VS] = Wec[:, lo:hi]
        whi = wp.astype(bf16)
        wlo = (wp - whi.astype(np.float32)).astype(bf16)
        becp = np.full((VSP,), NEG, np.float32)
        becp[:VS] = bec[lo:hi]
        in_maps.append({
            "x": x,
            "x_ids": ids,
            "Wq": Wq, "bq": bq, "Wk": Wk, "bk": bk, "Wv": Wv,
            "whi0": np.ascontiguousarray(whi[0:D]),
            "wlo0": np.ascontiguousarray(wlo[0:D]),
            "whi1": np.ascontiguousarray(whi[D:2 * D]),
            "wlo1": np.ascontiguousarray(wlo[D:2 * D]),
            "becp": becp,
            "lo_vec": np.full((128, 1), float(lo), np.float32),
        })

    from concourse.bass_utils import run_bass_kernel_spmd
    res = run_bass_kernel_spmd(nc, in_maps, core_ids=list(range(NCORES)))

    outp = np.empty((B, V), np.float32)
    for r in range(NCORES):
        o = res.results[r]["out"].reshape(128, B, NCHUNK)
        # out[p, b, c] -> probs[b, c*128 + p]
        shard = o.transpose(1, 2, 0).reshape(B, VSP)[:, :VS]
        outp[:, r * VS:(r + 1) * VS] = shard
    return outp
